# revision 67
# baseline (speedup 1.0000x reference)
"""BEiT-style relative-position-bias attention on 8 TRN2 NeuronCores.

Strategy: pure data-parallel over batch (64 / 8 = 8 per core, no collectives).
Host prep: transpose x and weights (to bf16), fold the 1/sqrt(D) scale into
W_q / q_bias, gather rel_pos_bias_table[rel_pos_index.T] into a dense
[H, N, N] bias.

Device (per core), software-pipelined over local batches: batch b's dense
qk^T / v matmuls are interleaved with batch b-1's attention heads so the PE
array duty cycle stays high (HAM stays un-throttled at 2.4 GHz).

  qk^T  [2048, 257] = W_qk^T-stationary matmuls over x^T[b]      (feature-major)
  v     [257, 1024] = x^T-stationary matmuls over W_v^T          (token-major)
  per head h:
    S^T  [257k, 257q] = k_h^T.T @ q_h^T  (+ bias via DVE add), exp on ACT
    ctx^T[65, 257] = [v_h | 1].T @ exp(S^T)   (row 64 = softmax denominators)
    normalize: reciprocal_approx_fast + gpsimd partition_broadcast + DVE mul
  out^T [1024, 257] = W_p^T-stationary matmuls over ctx^T, + bias, DMA out.
"""

import os
import numpy as np
from contextlib import ExitStack

B, N, C, H, D = 64, 257, 1024, 16, 64
NCORES = 8
BLOC = B // NCORES  # 8 batches per core
SCALE = D ** -0.5

LAST_EXEC_TIME_NS = None
LAST_RESULTS = None

_BUILT = None

TOK_TILES = [(0, 128), (128, 128), (256, 1)]


def build_kernel():
    import concourse.bacc as bacc
    import concourse.bass as bass
    import concourse.tile as tile
    from concourse import mybir
    from concourse.alu_op_type import AluOpType

    f32 = mybir.dt.float32
    bf16 = mybir.dt.bfloat16
    Act = mybir.ActivationFunctionType

    nc = bacc.Bacc(None)

    xt_d = nc.dram_tensor("xt", [BLOC, C, N], bf16, kind="ExternalInput")
    wqk_d = nc.dram_tensor("wqk", [C, 2 * C], bf16, kind="ExternalInput")
    wv_d = nc.dram_tensor("wv", [C, C], bf16, kind="ExternalInput")
    wp_d = nc.dram_tensor("wp", [C, C], bf16, kind="ExternalInput")
    qkb_d = nc.dram_tensor("qkb", [2 * C], f32, kind="ExternalInput")
    vb_d = nc.dram_tensor("vb", [C], f32, kind="ExternalInput")
    pb_d = nc.dram_tensor("pb", [C], f32, kind="ExternalInput")
    bt_d = nc.dram_tensor("bt", [H, N, N], bf16, kind="ExternalInput")  # [h, k, q]
    btr2_d = nc.dram_tensor("btr2", [H, N], f32, kind="ExternalInput")  # [h, q] k=256
    xtr_d = nc.dram_tensor("xtr", [C, BLOC], bf16, kind="ExternalInput")  # x[:,256,:]^T
    out_d = nc.dram_tensor("out", [BLOC, C, N], f32, kind="ExternalOutput")

    with tile.TileContext(nc) as tc:
        with ExitStack() as ctx:
            from concourse import library_config

            nc.gpsimd.load_library(library_config.mlp)
            singles = ctx.enter_context(tc.tile_pool(name="singles", bufs=1))
            xt_pool = ctx.enter_context(tc.tile_pool(name="xt", bufs=3))
            qk_pool = ctx.enter_context(tc.tile_pool(name="qk", bufs=2))
            v_pool = ctx.enter_context(tc.tile_pool(name="v", bufs=2))
            ctxa_pool = ctx.enter_context(tc.tile_pool(name="ctxa", bufs=3))
            outp_pool = ctx.enter_context(tc.tile_pool(name="outp", bufs=2))
            attn_pool = ctx.enter_context(tc.tile_pool(name="attn", bufs=10))
            rag_pool = ctx.enter_context(tc.tile_pool(name="rag", bufs=2))
            ei_pool = ctx.enter_context(tc.tile_pool(name="ei", bufs=6))
            small_pool = ctx.enter_context(tc.tile_pool(name="small", bufs=3))
            ps_pool = ctx.enter_context(
                tc.tile_pool(name="ps", bufs=8, space="PSUM")
            )

            # ---- persistent tiles. The input DMAs are chained into a
            # serial dependency order (first-use order): Tile spreads DMAs
            # across HW lanes so without the chain all ~14MB stream
            # concurrently and the first matmul's 1MB dependency starves. ----
            from concourse.tile_rust import add_dep_helper

            wqk_sb = singles.tile([128, 8, 2 * C], bf16)
            wv_sb = singles.tile([128, 8, C], bf16)
            wp_sb = singles.tile([128, 8, C], bf16)
            bt_sb = singles.tile([128, H, 2, N], bf16)
            btr2_sb = singles.tile([16, N], f32)
            qkb_sb = singles.tile([128, 16], f32)
            pb_sb = singles.tile([128, 8], f32)
            xtr_sb = singles.tile([128, 8, BLOC], bf16)
            vb_sb = singles.tile([128, C], f32)
            xt0_sb = xt_pool.tile([128, 8, N], bf16, tag="xt")
            xt0_dma = nc.sync.dma_start(
                out=xt0_sb[:], in_=xt_d[0].rearrange("(kt p) n -> p kt n", p=128)
            )
            vb_ap = vb_d[:]
            vb_bcast = bass.AP(
                tensor=vb_ap.tensor, offset=vb_ap.offset, ap=[[0, 128]] + list(vb_ap.ap)
            )
            nc.gpsimd.dma_start(out=vb_sb[:], in_=vb_bcast)

            def _wqk_ms(ms):
                return nc.sync.dma_start(
                    out=wqk_sb[:, :, ms * 512 : (ms + 1) * 512],
                    in_=wqk_d[:, ms * 512 : (ms + 1) * 512].rearrange(
                        "(kt p) f -> p kt f", p=128
                    ),
                )

            def _wv_ms(ms):
                return nc.sync.dma_start(
                    out=wv_sb[:, :, ms * 512 : (ms + 1) * 512],
                    in_=wv_d[:, ms * 512 : (ms + 1) * 512].rearrange(
                        "(kt p) f -> p kt f", p=128
                    ),
                )

            chain = [
                _wqk_ms(0),
                nc.sync.dma_start(
                    out=qkb_sb[:], in_=qkb_d[:].rearrange("(m p) -> p m", p=128)
                ),
                _wv_ms(0),
                _wqk_ms(1),
                _wv_ms(1),
                _wqk_ms(2),
                _wqk_ms(3),
                nc.sync.dma_start(
                    out=xtr_sb[:],
                    in_=xtr_d[:].rearrange("(kt p) b -> p kt b", p=128),
                ),
                nc.sync.dma_start(out=btr2_sb[:], in_=btr2_d[:]),
                nc.sync.dma_start(
                    out=bt_sb[:, :, 0, :],
                    in_=bt_d[:, 0:128, :].rearrange("h p q -> p h q"),
                ),
                nc.sync.dma_start(
                    out=bt_sb[:, :, 1, :],
                    in_=bt_d[:, 128:256, :].rearrange("h p q -> p h q"),
                ),
                nc.sync.dma_start(
                    out=wp_sb[:], in_=wp_d[:].rearrange("(kt p) f -> p kt f", p=128)
                ),
                nc.sync.dma_start(
                    out=pb_sb[:], in_=pb_d[:].rearrange("(m p) -> p m", p=128)
                ),
            ]
            # two-level ordering: mid-group DMAs wait for the critical first
            # wqk slice; late-group DMAs wait for the second wqk slice. Keeps
            # streaming pipelined within groups.
            add_dep_helper(chain[0].ins, xt0_dma.ins, reason="ms0 after xt0")
            for d in chain[2:7]:
                add_dep_helper(d.ins, chain[0].ins, reason="mid after ms0")
            for d in chain[7:]:
                add_dep_helper(d.ins, chain[3].ins, reason="late after ms1")
            first_chain_tail = chain[6]

            # ---------- emission helpers ----------
            def emit_qk_m(st, m):
                ps = ps_pool.tile([128, 512], f32, tag="ps")
                for kt in range(8):
                    nc.tensor.matmul(
                        ps[:, 0:N],
                        wqk_sb[:, kt, m * 128 : (m + 1) * 128],
                        st["xt"][:, kt, :],
                        start=(kt == 0),
                        stop=(kt == 7),
                    )
                nc.scalar.activation(
                    st["qk"][:, m, :], ps[:, 0:N], Act.Identity,
                    bias=qkb_sb[:, m : m + 1],
                )

            def emit_v_chunk(st, ci):
                mt, nch = divmod(ci, 2)
                t0, tn = TOK_TILES[mt]
                ps = ps_pool.tile([128, 512], f32, tag="ps")
                for kt in range(8):
                    nc.tensor.matmul(
                        ps[:tn, :],
                        st["xt"][:, kt, t0 : t0 + tn],
                        wv_sb[:, kt, nch * 512 : (nch + 1) * 512],
                        start=(kt == 0),
                        stop=(kt == 7),
                    )
                nc.vector.tensor_tensor(
                    st["v"][:tn, mt, nch * 8 : (nch + 1) * 8, 0:64],
                    ps[:tn, :].rearrange("p (h d) -> p h d", d=64),
                    vb_sb[:tn, nch * 512 : (nch + 1) * 512].rearrange(
                        "p (h d) -> p h d", d=64
                    ),
                    AluOpType.add,
                )

            def emit_ragged_batch(st):
                """All 16 heads' S rows for k-token 256 via one masked matmul
                chain: masked[:, t, h] = k_h^T[:, 256] (nonzero only in head
                h's 64-row slice of its feature tile t), so
                sum_t masked[:,t,:].T @ q^T[:,t,:] is block-diagonal per head.
                """
                qk_sb = st["qk"]
                masked = rag_pool.tile([128, 8, 16], bf16, tag="masked")
                nc.gpsimd.memset(masked[:], 0.0)
                for h in range(16):
                    hp = 64 * (h % 2)
                    ht = h // 2
                    nc.gpsimd.tensor_copy(
                        masked[hp : hp + 64, ht, h : h + 1],
                        qk_sb[hp : hp + 64, 8 + ht, 256:257],
                    )
                ps_rag = ps_pool.tile([16, 512], f32, tag="ps")
                for t in range(8):
                    nc.tensor.matmul(
                        ps_rag[:, 0:N],
                        masked[:, t, :],
                        qk_sb[:, t, :],
                        start=(t == 0),
                        stop=(t == 7),
                    )
                eirb = rag_pool.tile([16, N], f32, tag="eirb")
                nc.vector.tensor_tensor(
                    eirb[:], ps_rag[:, 0:N], btr2_sb[:], AluOpType.add
                )
                expr = rag_pool.tile([16, N], bf16, tag="expr")
                nc.scalar.activation(expr[:], eirb[:], Act.Exp)
                expr0 = rag_pool.tile([1, 16, N], bf16, tag="expr0")
                nc.sync.dma_start(out=expr0[:], in_=expr[:])
                st["expr0"] = expr0

            def emit_attn_S_pair(st, h0):
                """S matmuls for heads h0, h0+1 (even/odd → PE rows 0-63 /
                64-127) issued adjacently so they run concurrently."""
                qk_sb = st["qk"]
                ht = h0 // 2
                attns = [
                    attn_pool.tile([128, 2, N], bf16, tag="attn", name=f"at{h0}"),
                    attn_pool.tile([128, 2, N], bf16, tag="attn", name=f"at{h0 + 1}"),
                ]
                eis = [
                    ei_pool.tile([128, 2, N], f32, tag="ei", name=f"ei{h0}"),
                    ei_pool.tile([128, 2, N], f32, tag="ei", name=f"ei{h0 + 1}"),
                ]
                for kt in range(2):
                    pss = []
                    for j in range(2):
                        hp = 64 * j
                        ps_s = ps_pool.tile([128, 512], f32, tag="ps")
                        nc.tensor.matmul(
                            ps_s[:, 0:N],
                            qk_sb[hp : hp + 64, 8 + ht, kt * 128 : (kt + 1) * 128],
                            qk_sb[hp : hp + 64, ht, :],
                            start=True,
                            stop=True,
                        )
                        pss.append(ps_s)
                    for j in range(2):
                        nc.vector.tensor_tensor(
                            eis[j][:, kt, :], pss[j][:, 0:N], bt_sb[:, h0 + j, kt, :],
                            AluOpType.add,
                        )
                for j in range(2):
                    nc.scalar.activation(attns[j][:], eis[j][:], Act.Exp)
                return attns

            def emit_attn_ctx_mms(st, h, attn_sb, kt):
                """One stage of head h's ctx accumulation: kt=-1 ragged, 0, 1."""
                v_sb = st["v"]
                ps_c = st["ps_c"][h]
                if kt == -1:
                    nc.tensor.matmul(
                        ps_c[:, 0:N],
                        st["vr0"][0:1, h, :],
                        st["expr0"][0:1, h, :],
                        start=True,
                        stop=False,
                    )
                else:
                    nc.tensor.matmul(
                        ps_c[:, 0:N],
                        v_sb[:, kt, h, :],
                        attn_sb[:, kt, :],
                        start=False,
                        stop=(kt == 1),
                    )

            def emit_attn_norm(st, h):
                hp = 64 * (h % 2)
                ht = h // 2
                ps_c = st["ps_c"].pop(h)
                sums_sb = small_pool.tile([1, N], f32, tag="sums")
                nc.scalar.activation(sums_sb[:], ps_c[64:65, 0:N], Act.Copy)
                recip = small_pool.tile([1, N], f32, tag="recip")
                nc.vector.reciprocal_approx_fast(recip[:], sums_sb[:])
                bcast = small_pool.tile([64, N], f32, tag="bcast")
                nc.gpsimd.partition_broadcast(bcast[:], recip[:])
                nc.vector.tensor_tensor(
                    st["ctxa"][hp : hp + 64, ht, :],
                    ps_c[0:64, 0:N],
                    bcast[:],
                    AluOpType.mult,
                )

            def emit_proj(st):
                outp_sb = outp_pool.tile([128, 8, N], f32, tag="outp")
                for m in range(8):
                    ps = ps_pool.tile([128, 512], f32, tag="ps")
                    for kt in range(8):
                        nc.tensor.matmul(
                            ps[:, 0:N],
                            wp_sb[:, kt, m * 128 : (m + 1) * 128],
                            st["ctxa"][:, kt, :],
                            start=(kt == 0),
                            stop=(kt == 7),
                        )
                    nc.scalar.activation(
                        outp_sb[:, m, :], ps[:, 0:N], Act.Identity,
                        bias=pb_sb[:, m : m + 1],
                    )
                    if m == 3 or m == 7:
                        half = slice(m - 3, m + 1)
                        nc.sync.dma_start(
                            out=out_d[st["b"]]
                            .rearrange("(m p) n -> p m n", p=128)[:, half, :],
                            in_=outp_sb[:, half, :],
                        )

            # ---- ragged v rows for ALL batches in one matmul group ----
            vr_all = singles.tile([BLOC, C], bf16)

            def emit_vrall():
                for nch in range(2):
                    ps_vr = ps_pool.tile(
                        [BLOC, 512], f32, tag="ps", name=f"psvr{nch}"
                    )
                    for kt in range(8):
                        nc.tensor.matmul(
                            ps_vr[:, :],
                            xtr_sb[:, kt, :],
                            wv_sb[:, kt, nch * 512 : (nch + 1) * 512],
                            start=(kt == 0),
                            stop=(kt == 7),
                        )
                    nc.vector.tensor_tensor(
                        vr_all[:, nch * 512 : (nch + 1) * 512],
                        ps_vr[:, :],
                        vb_sb[0:BLOC, nch * 512 : (nch + 1) * 512],
                        AluOpType.add,
                    )

            # ---------- software pipeline over local batches ----------
            def emit_ctx_pair(st, h0, tiles):
                for j in range(2):
                    st["ps_c"][h0 + j] = ps_pool.tile(
                        [65, 512], f32, tag="ps", name=f"psc{h0 + j}"
                    )
                for j in range(2):
                    for kt in (-1, 0, 1):
                        emit_attn_ctx_mms(st, h0 + j, tiles[j], kt)
                emit_attn_norm(st, h0)
                emit_attn_norm(st, h0 + 1)

            xt_tiles = {0: xt0_sb}

            def prefetch_xt(b):
                if b < BLOC and b not in xt_tiles:
                    t = xt_pool.tile([128, 8, N], bf16, tag="xt", name=f"xt{b}")
                    d = nc.sync.dma_start(
                        out=t[:],
                        in_=xt_d[b].rearrange("(kt p) n -> p kt n", p=128),
                    )
                    if b == 1:
                        add_dep_helper(
                            d.ins, first_chain_tail.ins,
                            reason="xt1 after input stream",
                        )
                    xt_tiles[b] = t

            states = {}
            for step in range(BLOC + 1):
                prefetch_xt(step + 1)
                if step < BLOC:
                    b = step
                    v_sb = v_pool.tile([128, 2, 16, 65], bf16, tag="v")
                    nc.vector.memset(v_sb[:, :, :, 64:65], 1.0)
                    qk_sb = qk_pool.tile([128, 16, N], bf16, tag="qk", name=f"qk{b}")
                    states[b] = dict(
                        b=b, xt=xt_tiles[b], qk=qk_sb, v=v_sb, next_h0=0, pend=[]
                    )
                prev = states.get(step - 1)
                if prev is not None:
                    ctxa_sb = ctxa_pool.tile(
                        [128, 8, N], bf16, tag="ctxa", name=f"ctxa{step - 1}"
                    )
                    prev["ctxa"] = ctxa_sb
                    prev["ps_c"] = {}
                    pb_ = prev["b"]
                    vr0 = rag_pool.tile([1, 16, 65], bf16, tag="vr0", name=f"vr{pb_}")
                    nc.vector.memset(vr0[:, :, 64:65], 1.0)
                    nc.sync.dma_start(
                        out=vr0[:, :, 0:64],
                        in_=vr_all[pb_ : pb_ + 1, :].rearrange(
                            "o (h d) -> o h d", d=64
                        ),
                    )
                    prev["vr0"] = vr0
                    emit_ragged_batch(prev)
                vq = 0  # v chunks emitted
                pend = states.get(step - 1, {}).get("pend", []) if prev else []
                for i in range(16):
                    if step < BLOC:
                        emit_qk_m(states[step], i)
                        if step == 0:
                            if i in (3, 5, 7, 9) and vq < 4:
                                emit_v_chunk(states[step], vq)
                                vq += 1
                        elif i % 3 == 2 and vq < 4:
                            emit_v_chunk(states[step], vq)
                            vq += 1
                    if prev is not None and i % 2 == 1 and prev["next_h0"] < 16:
                        h0 = prev["next_h0"]
                        prev["next_h0"] += 2
                        tiles = emit_attn_S_pair(prev, h0)
                        pend.append((h0, tiles))
                        if len(pend) > 2:
                            pp = pend.pop(0)
                            emit_ctx_pair(prev, pp[0], pp[1])
                    # pull the last batch's first S-pairs into its own
                    # iteration (its qk tiles are ready mid-loop) so the
                    # attention-only tail iteration is shorter
                    if step == BLOC - 1 and i in (11, 13, 15):
                        st7 = states[step]
                        j2 = st7["next_h0"]
                        st7["next_h0"] += 2
                        tiles = emit_attn_S_pair(st7, j2)
                        st7["pend"].append((j2, tiles))
                if step < BLOC:
                    while vq < 4:
                        emit_v_chunk(states[step], vq)
                        vq += 1
                    if step == 0:
                        emit_vrall()
                if prev is not None:
                    for pp in pend:
                        emit_ctx_pair(prev, pp[0], pp[1])
                    emit_proj(prev)
                    del states[step - 1]
                if step < BLOC:
                    del xt_tiles[step]  # release reference (pool handles reuse)

    nc.finalize()
    return nc


def _prep_inputs(x, qkv_weight, query_bias, value_bias, rel_pos_bias_table,
                 proj_weight, proj_bias, rel_pos_index):
    import ml_dtypes

    bf16 = ml_dtypes.bfloat16
    x = np.asarray(x, np.float32)
    wqkv = np.asarray(qkv_weight, np.float32)
    qb = np.asarray(query_bias, np.float32)
    vb = np.asarray(value_bias, np.float32)
    table = np.asarray(rel_pos_bias_table, np.float32)
    wp = np.asarray(proj_weight, np.float32)
    pb = np.asarray(proj_bias, np.float32)
    idx = np.asarray(rel_pos_index, np.int64)

    wq = wqkv[:C] * SCALE
    wk = wqkv[C : 2 * C]
    wv = wqkv[2 * C :]
    wqk_t = np.ascontiguousarray(np.concatenate([wq, wk], 0).T).astype(bf16)
    wv_t = np.ascontiguousarray(wv.T).astype(bf16)  # [C, C]
    wp_t = np.ascontiguousarray(wp.T).astype(bf16)  # [C, C]
    qkb = np.concatenate([qb * SCALE, np.zeros(C, np.float32)])
    # bias^T[h, k, q] = table[idx[q, k], h]
    bt = np.ascontiguousarray(table[idx.T].transpose(2, 0, 1)).astype(bf16)  # [H, N, N]
    btr2 = np.ascontiguousarray(bt[:, 256, :]).astype(np.float32)  # [H, N] ragged k row
    # x shards: [core, bloc, C, N]
    shards = np.ascontiguousarray(
        x.reshape(NCORES, BLOC, N, C).transpose(0, 1, 3, 2)
    ).astype(bf16)
    shared = dict(
        wqk=wqk_t, wv=wv_t, wp=wp_t, qkb=qkb, vb=vb, pb=pb, bt=bt, btr2=btr2
    )
    in_maps = [
        dict(
            shared,
            xt=shards[i],
            xtr=np.ascontiguousarray(shards[i][:, :, 256].T),
        )
        for i in range(NCORES)
    ]
    return in_maps


def _ensure_ntff_hook():
    """Install antenv.axon_hooks with a ctypes NTFF profile hook if missing.

    Mirrors trn_agent_boot.trn_boot's hook so run_bass_kernel_spmd(trace=True)
    can capture NTFF profiles under axon. No-op if the module already exists.
    """
    import sys
    import types
    import ctypes
    import contextlib

    try:
        from antenv.axon_hooks import get_axon_ntff_profile_hook  # noqa: F401
        return
    except ImportError:
        pass
    so_path = "/opt/axon/libaxon_pjrt.so"
    if not os.path.exists(so_path):
        return
    lib = ctypes.CDLL(so_path)
    if not hasattr(lib, "axon_start_nrt_profile"):
        return
    lib.axon_start_nrt_profile.argtypes = [
        ctypes.POINTER(ctypes.c_int64),
        ctypes.c_size_t,
    ]
    lib.axon_start_nrt_profile.restype = ctypes.c_int64
    lib.axon_stop_nrt_profile.argtypes = [ctypes.c_char_p]
    lib.axon_stop_nrt_profile.restype = ctypes.c_int64

    @contextlib.contextmanager
    def _hook(output_dir, device_ids):
        import jax

        jax.devices()
        if device_ids:
            ids = (ctypes.c_int64 * len(device_ids))(*device_ids)
            rc = lib.axon_start_nrt_profile(ids, len(device_ids))
        else:
            rc = lib.axon_start_nrt_profile(None, 0)
        if rc != 0:
            raise RuntimeError(f"axon_start_nrt_profile rc={rc}")
        try:
            yield
        finally:
            n = lib.axon_stop_nrt_profile(str(output_dir).encode())
            if n < 0:
                raise RuntimeError(f"axon_stop_nrt_profile rc={n}")
            print(f"profile: {n} file(s) written to {output_dir}")

    mod = types.ModuleType("antenv.axon_hooks")
    _state = {"hook": _hook}
    mod.set_axon_ntff_profile_hook = lambda h: _state.__setitem__("hook", h)
    mod.get_axon_ntff_profile_hook = lambda: _state["hook"]
    import antenv

    antenv.axon_hooks = mod
    sys.modules["antenv.axon_hooks"] = mod


def kernel(x, qkv_weight, query_bias, value_bias, rel_pos_bias_table,
           proj_weight, proj_bias, rel_pos_index):
    global LAST_EXEC_TIME_NS, LAST_RESULTS, _BUILT
    from concourse.bass_utils import run_bass_kernel_spmd

    in_maps = _prep_inputs(
        x, qkv_weight, query_bias, value_bias, rel_pos_bias_table,
        proj_weight, proj_bias, rel_pos_index,
    )
    if _BUILT is None:
        _BUILT = build_kernel()
    nc = _BUILT
    trace = bool(os.environ.get("BASS_TRACE"))
    if trace:
        _ensure_ntff_hook()
    res = None
    last_exc = None
    for attempt in range(3):
        try:
            res = run_bass_kernel_spmd(nc, in_maps, list(range(NCORES)), trace=trace)
            break
        except Exception as e:  # transient NRT device errors recover on retry
            last_exc = e
            import time

            time.sleep(5)
    if res is None:
        raise last_exc
    LAST_RESULTS = res
    LAST_EXEC_TIME_NS = res.exec_time_ns
    if res.exec_time_ns is not None:
        print(f"HW exec time: {res.exec_time_ns} ns")
    out_t = np.stack([res.results[i]["out"] for i in range(NCORES)])  # [8, BLOC, C, N]
    out = out_t.reshape(B, C, N).transpose(0, 2, 1)
    return np.ascontiguousarray(out.astype(np.float32))


# revision 68
# speedup vs baseline: 1.0192x; 1.0192x over previous
"""BEiT-style relative-position-bias attention on 8 TRN2 NeuronCores.

Strategy: pure data-parallel over batch (64 / 8 = 8 per core, no collectives).
Host prep: transpose x and weights (to bf16), fold the 1/sqrt(D) scale into
W_q / q_bias, gather rel_pos_bias_table[rel_pos_index.T] into a dense
[H, N, N] bias.

Device (per core), software-pipelined over local batches: batch b's dense
qk^T / v matmuls are interleaved with batch b-1's attention heads so the PE
array duty cycle stays high (HAM stays un-throttled at 2.4 GHz).

  qk^T  [2048, 257] = W_qk^T-stationary matmuls over x^T[b]      (feature-major)
  v     [257, 1024] = x^T-stationary matmuls over W_v^T          (token-major)
  per head h:
    S^T  [257k, 257q] = k_h^T.T @ q_h^T  (+ bias via DVE add), exp on ACT
    ctx^T[65, 257] = [v_h | 1].T @ exp(S^T)   (row 64 = softmax denominators)
    normalize: reciprocal_approx_fast + gpsimd partition_broadcast + DVE mul
  out^T [1024, 257] = W_p^T-stationary matmuls over ctx^T, + bias, DMA out.
"""

import os
import numpy as np
from contextlib import ExitStack

B, N, C, H, D = 64, 257, 1024, 16, 64
NCORES = 8
BLOC = B // NCORES  # 8 batches per core
SCALE = D ** -0.5

LAST_EXEC_TIME_NS = None
LAST_RESULTS = None

_BUILT = None

TOK_TILES = [(0, 128), (128, 128), (256, 1)]


def build_kernel():
    import concourse.bacc as bacc
    import concourse.bass as bass
    import concourse.tile as tile
    from concourse import mybir
    from concourse.alu_op_type import AluOpType

    f32 = mybir.dt.float32
    bf16 = mybir.dt.bfloat16
    Act = mybir.ActivationFunctionType

    nc = bacc.Bacc(None)

    xt_d = nc.dram_tensor("xt", [BLOC, C, N], bf16, kind="ExternalInput")
    wqk_d = nc.dram_tensor("wqk", [C, 2 * C], bf16, kind="ExternalInput")
    wv_d = nc.dram_tensor("wv", [C, C], bf16, kind="ExternalInput")
    wp_d = nc.dram_tensor("wp", [C, C], bf16, kind="ExternalInput")
    qkb_d = nc.dram_tensor("qkb", [2 * C], f32, kind="ExternalInput")
    vb_d = nc.dram_tensor("vb", [C], f32, kind="ExternalInput")
    pb_d = nc.dram_tensor("pb", [C], f32, kind="ExternalInput")
    bt_d = nc.dram_tensor("bt", [H, N, N], bf16, kind="ExternalInput")  # [h, k, q]
    btr2_d = nc.dram_tensor("btr2", [H, N], f32, kind="ExternalInput")  # [h, q] k=256
    xtr_d = nc.dram_tensor("xtr", [C, BLOC], bf16, kind="ExternalInput")  # x[:,256,:]^T
    out_d = nc.dram_tensor("out", [BLOC, C, N], f32, kind="ExternalOutput")

    with tile.TileContext(nc) as tc:
        with ExitStack() as ctx:
            from concourse import library_config

            nc.gpsimd.load_library(library_config.mlp)
            singles = ctx.enter_context(tc.tile_pool(name="singles", bufs=1))
            xt_pool = ctx.enter_context(tc.tile_pool(name="xt", bufs=3))
            qk_pool = ctx.enter_context(tc.tile_pool(name="qk", bufs=2))
            v_pool = ctx.enter_context(tc.tile_pool(name="v", bufs=2))
            ctxa_pool = ctx.enter_context(tc.tile_pool(name="ctxa", bufs=3))
            outp_pool = ctx.enter_context(tc.tile_pool(name="outp", bufs=2))
            attn_pool = ctx.enter_context(tc.tile_pool(name="attn", bufs=10))
            rag_pool = ctx.enter_context(tc.tile_pool(name="rag", bufs=2))
            ei_pool = ctx.enter_context(tc.tile_pool(name="ei", bufs=6))
            small_pool = ctx.enter_context(tc.tile_pool(name="small", bufs=3))
            ps_pool = ctx.enter_context(
                tc.tile_pool(name="ps", bufs=8, space="PSUM")
            )

            # ---- persistent tiles. The input DMAs are chained into a
            # serial dependency order (first-use order): Tile spreads DMAs
            # across HW lanes so without the chain all ~14MB stream
            # concurrently and the first matmul's 1MB dependency starves. ----
            from concourse.tile_rust import add_dep_helper

            wqk_sb = singles.tile([128, 8, 2 * C], bf16)
            wv_sb = singles.tile([128, 8, C], bf16)
            wp_sb = singles.tile([128, 8, C], bf16)
            bt_sb = singles.tile([128, H, 2, N], bf16)
            btr2_sb = singles.tile([16, N], f32)
            qkb_sb = singles.tile([128, 16], f32)
            pb_sb = singles.tile([128, 8], f32)
            xtr_sb = singles.tile([128, 8, BLOC], bf16)
            vb_sb = singles.tile([128, C], f32)
            xt0_sb = xt_pool.tile([128, 8, N], bf16, tag="xt")
            xt0_dma = nc.sync.dma_start(
                out=xt0_sb[:], in_=xt_d[0].rearrange("(kt p) n -> p kt n", p=128)
            )
            vb_ap = vb_d[:]
            vb_bcast = bass.AP(
                tensor=vb_ap.tensor, offset=vb_ap.offset, ap=[[0, 128]] + list(vb_ap.ap)
            )
            nc.gpsimd.dma_start(out=vb_sb[:], in_=vb_bcast)

            def _wqk_ms(ms):
                return nc.sync.dma_start(
                    out=wqk_sb[:, :, ms * 512 : (ms + 1) * 512],
                    in_=wqk_d[:, ms * 512 : (ms + 1) * 512].rearrange(
                        "(kt p) f -> p kt f", p=128
                    ),
                )

            def _wv_ms(ms):
                return nc.sync.dma_start(
                    out=wv_sb[:, :, ms * 512 : (ms + 1) * 512],
                    in_=wv_d[:, ms * 512 : (ms + 1) * 512].rearrange(
                        "(kt p) f -> p kt f", p=128
                    ),
                )

            chain = [
                _wqk_ms(0),
                nc.sync.dma_start(
                    out=qkb_sb[:], in_=qkb_d[:].rearrange("(m p) -> p m", p=128)
                ),
                _wv_ms(0),
                _wqk_ms(1),
                _wv_ms(1),
                _wqk_ms(2),
                _wqk_ms(3),
                nc.sync.dma_start(
                    out=xtr_sb[:],
                    in_=xtr_d[:].rearrange("(kt p) b -> p kt b", p=128),
                ),
                nc.sync.dma_start(out=btr2_sb[:], in_=btr2_d[:]),
                nc.sync.dma_start(
                    out=bt_sb[:, :, 0, :],
                    in_=bt_d[:, 0:128, :].rearrange("h p q -> p h q"),
                ),
                nc.sync.dma_start(
                    out=bt_sb[:, :, 1, :],
                    in_=bt_d[:, 128:256, :].rearrange("h p q -> p h q"),
                ),
                nc.sync.dma_start(
                    out=wp_sb[:], in_=wp_d[:].rearrange("(kt p) f -> p kt f", p=128)
                ),
                nc.sync.dma_start(
                    out=pb_sb[:], in_=pb_d[:].rearrange("(m p) -> p m", p=128)
                ),
            ]
            # two-level ordering: mid-group DMAs wait for the critical first
            # wqk slice; late-group DMAs wait for the second wqk slice. Keeps
            # streaming pipelined within groups.
            add_dep_helper(chain[0].ins, xt0_dma.ins, reason="ms0 after xt0")
            for d in chain[2:7]:
                add_dep_helper(d.ins, chain[0].ins, reason="mid after ms0")
            for d in chain[7:]:
                add_dep_helper(d.ins, chain[3].ins, reason="late after ms1")
            first_chain_tail = chain[6]

            # ---------- emission helpers ----------
            def emit_qk_m(st, m):
                ps = ps_pool.tile([128, 512], f32, tag="ps")
                for kt in range(8):
                    nc.tensor.matmul(
                        ps[:, 0:N],
                        wqk_sb[:, kt, m * 128 : (m + 1) * 128],
                        st["xt"][:, kt, :],
                        start=(kt == 0),
                        stop=(kt == 7),
                    )
                nc.scalar.activation(
                    st["qk"][:, m, :], ps[:, 0:N], Act.Identity,
                    bias=qkb_sb[:, m : m + 1],
                )

            def emit_v_chunk(st, ci):
                mt, nch = divmod(ci, 2)
                t0, tn = TOK_TILES[mt]
                ps = ps_pool.tile([128, 512], f32, tag="ps")
                for kt in range(8):
                    nc.tensor.matmul(
                        ps[:tn, :],
                        st["xt"][:, kt, t0 : t0 + tn],
                        wv_sb[:, kt, nch * 512 : (nch + 1) * 512],
                        start=(kt == 0),
                        stop=(kt == 7),
                    )
                nc.vector.tensor_tensor(
                    st["v"][:tn, mt, nch * 8 : (nch + 1) * 8, 0:64],
                    ps[:tn, :].rearrange("p (h d) -> p h d", d=64),
                    vb_sb[:tn, nch * 512 : (nch + 1) * 512].rearrange(
                        "p (h d) -> p h d", d=64
                    ),
                    AluOpType.add,
                )

            def emit_ragged_batch(st):
                """All 16 heads' S rows for k-token 256 via one masked matmul
                chain: masked[:, t, h] = k_h^T[:, 256] (nonzero only in head
                h's 64-row slice of its feature tile t), so
                sum_t masked[:,t,:].T @ q^T[:,t,:] is block-diagonal per head.
                """
                qk_sb = st["qk"]
                masked = rag_pool.tile([128, 8, 16], bf16, tag="masked")
                nc.gpsimd.memset(masked[:], 0.0)
                for h in range(16):
                    hp = 64 * (h % 2)
                    ht = h // 2
                    nc.gpsimd.tensor_copy(
                        masked[hp : hp + 64, ht, h : h + 1],
                        qk_sb[hp : hp + 64, 8 + ht, 256:257],
                    )
                ps_rag = ps_pool.tile([16, 512], f32, tag="ps")
                for t in range(8):
                    nc.tensor.matmul(
                        ps_rag[:, 0:N],
                        masked[:, t, :],
                        qk_sb[:, t, :],
                        start=(t == 0),
                        stop=(t == 7),
                    )
                eirb = rag_pool.tile([16, N], f32, tag="eirb")
                nc.vector.tensor_tensor(
                    eirb[:], ps_rag[:, 0:N], btr2_sb[:], AluOpType.add
                )
                expr = rag_pool.tile([16, N], bf16, tag="expr")
                nc.scalar.activation(expr[:], eirb[:], Act.Exp)
                expr0 = rag_pool.tile([1, 16, N], bf16, tag="expr0")
                nc.sync.dma_start(out=expr0[:], in_=expr[:])
                st["expr0"] = expr0

            def emit_attn_S_pair(st, h0):
                """S matmuls for heads h0, h0+1 (even/odd → PE rows 0-63 /
                64-127) issued adjacently so they run concurrently."""
                qk_sb = st["qk"]
                ht = h0 // 2
                attns = [
                    attn_pool.tile([128, 2, N], bf16, tag="attn", name=f"at{h0}"),
                    attn_pool.tile([128, 2, N], bf16, tag="attn", name=f"at{h0 + 1}"),
                ]
                eis = [
                    ei_pool.tile([128, 2, N], f32, tag="ei", name=f"ei{h0}"),
                    ei_pool.tile([128, 2, N], f32, tag="ei", name=f"ei{h0 + 1}"),
                ]
                for kt in range(2):
                    pss = []
                    for j in range(2):
                        hp = 64 * j
                        ps_s = ps_pool.tile([128, 512], f32, tag="ps")
                        nc.tensor.matmul(
                            ps_s[:, 0:N],
                            qk_sb[hp : hp + 64, 8 + ht, kt * 128 : (kt + 1) * 128],
                            qk_sb[hp : hp + 64, ht, :],
                            start=True,
                            stop=True,
                        )
                        pss.append(ps_s)
                    for j in range(2):
                        nc.vector.tensor_tensor(
                            eis[j][:, kt, :], pss[j][:, 0:N], bt_sb[:, h0 + j, kt, :],
                            AluOpType.add,
                        )
                for j in range(2):
                    nc.scalar.activation(attns[j][:], eis[j][:], Act.Exp)
                return attns

            def emit_attn_ctx_mms(st, h, attn_sb, kt):
                """One stage of head h's ctx accumulation: kt=-1 ragged, 0, 1."""
                v_sb = st["v"]
                ps_c = st["ps_c"][h]
                if kt == -1:
                    nc.tensor.matmul(
                        ps_c[:, 0:N],
                        st["vr0"][0:1, h, :],
                        st["expr0"][0:1, h, :],
                        start=True,
                        stop=False,
                    )
                else:
                    nc.tensor.matmul(
                        ps_c[:, 0:N],
                        v_sb[:, kt, h, :],
                        attn_sb[:, kt, :],
                        start=False,
                        stop=(kt == 1),
                    )

            def emit_attn_norm(st, h):
                hp = 64 * (h % 2)
                ht = h // 2
                ps_c = st["ps_c"].pop(h)
                sums_sb = small_pool.tile([1, N], f32, tag="sums")
                nc.scalar.activation(sums_sb[:], ps_c[64:65, 0:N], Act.Copy)
                recip = small_pool.tile([1, N], f32, tag="recip")
                nc.vector.reciprocal_approx_fast(recip[:], sums_sb[:])
                bcast = small_pool.tile([64, N], f32, tag="bcast")
                nc.gpsimd.partition_broadcast(bcast[:], recip[:])
                nc.vector.tensor_tensor(
                    st["ctxa"][hp : hp + 64, ht, :],
                    ps_c[0:64, 0:N],
                    bcast[:],
                    AluOpType.mult,
                )

            def emit_proj(st):
                outp_sb = outp_pool.tile([128, 8, N], f32, tag="outp")
                for m in range(8):
                    ps = ps_pool.tile([128, 512], f32, tag="ps")
                    for kt in range(8):
                        nc.tensor.matmul(
                            ps[:, 0:N],
                            wp_sb[:, kt, m * 128 : (m + 1) * 128],
                            st["ctxa"][:, kt, :],
                            start=(kt == 0),
                            stop=(kt == 7),
                        )
                    nc.scalar.activation(
                        outp_sb[:, m, :], ps[:, 0:N], Act.Identity,
                        bias=pb_sb[:, m : m + 1],
                    )
                    if m == 3 or m == 7:
                        half = slice(m - 3, m + 1)
                        nc.sync.dma_start(
                            out=out_d[st["b"]]
                            .rearrange("(m p) n -> p m n", p=128)[:, half, :],
                            in_=outp_sb[:, half, :],
                        )

            # ---- ragged v rows for ALL batches in one matmul group ----
            vr_all = singles.tile([BLOC, C], bf16)

            def emit_vrall():
                for nch in range(2):
                    ps_vr = ps_pool.tile(
                        [BLOC, 512], f32, tag="ps", name=f"psvr{nch}"
                    )
                    for kt in range(8):
                        nc.tensor.matmul(
                            ps_vr[:, :],
                            xtr_sb[:, kt, :],
                            wv_sb[:, kt, nch * 512 : (nch + 1) * 512],
                            start=(kt == 0),
                            stop=(kt == 7),
                        )
                    nc.vector.tensor_tensor(
                        vr_all[:, nch * 512 : (nch + 1) * 512],
                        ps_vr[:, :],
                        vb_sb[0:BLOC, nch * 512 : (nch + 1) * 512],
                        AluOpType.add,
                    )

            # ---------- software pipeline over local batches ----------
            def emit_ctx_pair(st, h0, tiles):
                for j in range(2):
                    st["ps_c"][h0 + j] = ps_pool.tile(
                        [65, 512], f32, tag="ps", name=f"psc{h0 + j}"
                    )
                for kt in (-1, 0, 1):
                    for j in range(2):
                        emit_attn_ctx_mms(st, h0 + j, tiles[j], kt)
                emit_attn_norm(st, h0)
                emit_attn_norm(st, h0 + 1)

            xt_tiles = {0: xt0_sb}

            def prefetch_xt(b):
                if b < BLOC and b not in xt_tiles:
                    t = xt_pool.tile([128, 8, N], bf16, tag="xt", name=f"xt{b}")
                    d = nc.sync.dma_start(
                        out=t[:],
                        in_=xt_d[b].rearrange("(kt p) n -> p kt n", p=128),
                    )
                    if b == 1:
                        add_dep_helper(
                            d.ins, first_chain_tail.ins,
                            reason="xt1 after input stream",
                        )
                    xt_tiles[b] = t

            states = {}
            for step in range(BLOC + 1):
                prefetch_xt(step + 1)
                if step < BLOC:
                    b = step
                    v_sb = v_pool.tile([128, 2, 16, 65], bf16, tag="v")
                    nc.vector.memset(v_sb[:, :, :, 64:65], 1.0)
                    qk_sb = qk_pool.tile([128, 16, N], bf16, tag="qk", name=f"qk{b}")
                    states[b] = dict(
                        b=b, xt=xt_tiles[b], qk=qk_sb, v=v_sb, next_h0=0, pend=[]
                    )
                prev = states.get(step - 1)
                if prev is not None:
                    ctxa_sb = ctxa_pool.tile(
                        [128, 8, N], bf16, tag="ctxa", name=f"ctxa{step - 1}"
                    )
                    prev["ctxa"] = ctxa_sb
                    prev["ps_c"] = {}
                    pb_ = prev["b"]
                    vr0 = rag_pool.tile([1, 16, 65], bf16, tag="vr0", name=f"vr{pb_}")
                    nc.vector.memset(vr0[:, :, 64:65], 1.0)
                    nc.sync.dma_start(
                        out=vr0[:, :, 0:64],
                        in_=vr_all[pb_ : pb_ + 1, :].rearrange(
                            "o (h d) -> o h d", d=64
                        ),
                    )
                    prev["vr0"] = vr0
                    emit_ragged_batch(prev)
                vq = 0  # v chunks emitted
                pend = states.get(step - 1, {}).get("pend", []) if prev else []
                for i in range(16):
                    if step < BLOC:
                        emit_qk_m(states[step], i)
                        if step == 0:
                            if i in (3, 5, 7, 9) and vq < 4:
                                emit_v_chunk(states[step], vq)
                                vq += 1
                        elif i % 3 == 2 and vq < 4:
                            emit_v_chunk(states[step], vq)
                            vq += 1
                    if prev is not None and i % 2 == 1 and prev["next_h0"] < 16:
                        h0 = prev["next_h0"]
                        prev["next_h0"] += 2
                        tiles = emit_attn_S_pair(prev, h0)
                        pend.append((h0, tiles))
                        if len(pend) > 2:
                            pp = pend.pop(0)
                            emit_ctx_pair(prev, pp[0], pp[1])
                    # pull the last batch's first S-pairs into its own
                    # iteration (its qk tiles are ready mid-loop) so the
                    # attention-only tail iteration is shorter
                    if step == BLOC - 1 and i in (11, 13, 15):
                        st7 = states[step]
                        j2 = st7["next_h0"]
                        st7["next_h0"] += 2
                        tiles = emit_attn_S_pair(st7, j2)
                        st7["pend"].append((j2, tiles))
                if step < BLOC:
                    while vq < 4:
                        emit_v_chunk(states[step], vq)
                        vq += 1
                    if step == 0:
                        emit_vrall()
                if prev is not None:
                    for pp in pend:
                        emit_ctx_pair(prev, pp[0], pp[1])
                    emit_proj(prev)
                    del states[step - 1]
                if step < BLOC:
                    del xt_tiles[step]  # release reference (pool handles reuse)

    nc.finalize()
    return nc


def _prep_inputs(x, qkv_weight, query_bias, value_bias, rel_pos_bias_table,
                 proj_weight, proj_bias, rel_pos_index):
    import ml_dtypes

    bf16 = ml_dtypes.bfloat16
    x = np.asarray(x, np.float32)
    wqkv = np.asarray(qkv_weight, np.float32)
    qb = np.asarray(query_bias, np.float32)
    vb = np.asarray(value_bias, np.float32)
    table = np.asarray(rel_pos_bias_table, np.float32)
    wp = np.asarray(proj_weight, np.float32)
    pb = np.asarray(proj_bias, np.float32)
    idx = np.asarray(rel_pos_index, np.int64)

    wq = wqkv[:C] * SCALE
    wk = wqkv[C : 2 * C]
    wv = wqkv[2 * C :]
    wqk_t = np.ascontiguousarray(np.concatenate([wq, wk], 0).T).astype(bf16)
    wv_t = np.ascontiguousarray(wv.T).astype(bf16)  # [C, C]
    wp_t = np.ascontiguousarray(wp.T).astype(bf16)  # [C, C]
    qkb = np.concatenate([qb * SCALE, np.zeros(C, np.float32)])
    # bias^T[h, k, q] = table[idx[q, k], h]
    bt = np.ascontiguousarray(table[idx.T].transpose(2, 0, 1)).astype(bf16)  # [H, N, N]
    btr2 = np.ascontiguousarray(bt[:, 256, :]).astype(np.float32)  # [H, N] ragged k row
    # x shards: [core, bloc, C, N]
    shards = np.ascontiguousarray(
        x.reshape(NCORES, BLOC, N, C).transpose(0, 1, 3, 2)
    ).astype(bf16)
    shared = dict(
        wqk=wqk_t, wv=wv_t, wp=wp_t, qkb=qkb, vb=vb, pb=pb, bt=bt, btr2=btr2
    )
    in_maps = [
        dict(
            shared,
            xt=shards[i],
            xtr=np.ascontiguousarray(shards[i][:, :, 256].T),
        )
        for i in range(NCORES)
    ]
    return in_maps


def _ensure_ntff_hook():
    """Install antenv.axon_hooks with a ctypes NTFF profile hook if missing.

    Mirrors trn_agent_boot.trn_boot's hook so run_bass_kernel_spmd(trace=True)
    can capture NTFF profiles under axon. No-op if the module already exists.
    """
    import sys
    import types
    import ctypes
    import contextlib

    try:
        from antenv.axon_hooks import get_axon_ntff_profile_hook  # noqa: F401
        return
    except ImportError:
        pass
    so_path = "/opt/axon/libaxon_pjrt.so"
    if not os.path.exists(so_path):
        return
    lib = ctypes.CDLL(so_path)
    if not hasattr(lib, "axon_start_nrt_profile"):
        return
    lib.axon_start_nrt_profile.argtypes = [
        ctypes.POINTER(ctypes.c_int64),
        ctypes.c_size_t,
    ]
    lib.axon_start_nrt_profile.restype = ctypes.c_int64
    lib.axon_stop_nrt_profile.argtypes = [ctypes.c_char_p]
    lib.axon_stop_nrt_profile.restype = ctypes.c_int64

    @contextlib.contextmanager
    def _hook(output_dir, device_ids):
        import jax

        jax.devices()
        if device_ids:
            ids = (ctypes.c_int64 * len(device_ids))(*device_ids)
            rc = lib.axon_start_nrt_profile(ids, len(device_ids))
        else:
            rc = lib.axon_start_nrt_profile(None, 0)
        if rc != 0:
            raise RuntimeError(f"axon_start_nrt_profile rc={rc}")
        try:
            yield
        finally:
            n = lib.axon_stop_nrt_profile(str(output_dir).encode())
            if n < 0:
                raise RuntimeError(f"axon_stop_nrt_profile rc={n}")
            print(f"profile: {n} file(s) written to {output_dir}")

    mod = types.ModuleType("antenv.axon_hooks")
    _state = {"hook": _hook}
    mod.set_axon_ntff_profile_hook = lambda h: _state.__setitem__("hook", h)
    mod.get_axon_ntff_profile_hook = lambda: _state["hook"]
    import antenv

    antenv.axon_hooks = mod
    sys.modules["antenv.axon_hooks"] = mod


def kernel(x, qkv_weight, query_bias, value_bias, rel_pos_bias_table,
           proj_weight, proj_bias, rel_pos_index):
    global LAST_EXEC_TIME_NS, LAST_RESULTS, _BUILT
    from concourse.bass_utils import run_bass_kernel_spmd

    in_maps = _prep_inputs(
        x, qkv_weight, query_bias, value_bias, rel_pos_bias_table,
        proj_weight, proj_bias, rel_pos_index,
    )
    if _BUILT is None:
        _BUILT = build_kernel()
    nc = _BUILT
    trace = bool(os.environ.get("BASS_TRACE"))
    if trace:
        _ensure_ntff_hook()
    res = None
    last_exc = None
    for attempt in range(3):
        try:
            res = run_bass_kernel_spmd(nc, in_maps, list(range(NCORES)), trace=trace)
            break
        except Exception as e:  # transient NRT device errors recover on retry
            last_exc = e
            import time

            time.sleep(5)
    if res is None:
        raise last_exc
    LAST_RESULTS = res
    LAST_EXEC_TIME_NS = res.exec_time_ns
    if res.exec_time_ns is not None:
        print(f"HW exec time: {res.exec_time_ns} ns")
    out_t = np.stack([res.results[i]["out"] for i in range(NCORES)])  # [8, BLOC, C, N]
    out = out_t.reshape(B, C, N).transpose(0, 2, 1)
    return np.ascontiguousarray(out.astype(np.float32))
